# revision 9
# baseline (speedup 1.0000x reference)
"""TRN2 Bass kernel for nn_Attention_24309514895857.

Multi-head attention (16 heads, dim_head 128, d_model 2048, b=2, n=2048) with
rotary embedding, sharded tensor-parallel over 8 NeuronCores: 2 heads per core.
Each core computes q/k/v projections for its heads, rotary, softmax attention,
and its partial contribution to the output projection (row-parallel Wo). The
host sums the 8 partials (the row-parallel unshard) and adds the bias.

All matmuls run in float32r (TF32-like single-pass fp32, full PE rate).
Everything on-device is feature-major ("transposed") so no transposes are
needed: x arrives as xT (d_model, tokens), q/k live as (dim_head, tokens),
attention scores as (k_tok, q_tok), output partial leaves as yT (d_model, tok).

rotate_half is a fixed pair-swap permutation of the dim_head axis -> done with
a 128x128 permutation matmul on the PE; the sign and the 1/sqrt(d) scale are
folded into host-precomputed sin/cos tables and Wq respectively.

Softmax skips the max-subtraction (logits are ~N(0,1) here; exp is safe) so
the denominator comes from an all-ones matmul that also broadcasts the sums
across all 128 partitions for the normalization divide.
"""

import numpy as np

HEADS = 16
DH = 128          # dim_head
D = 2048          # d_model
B = 2
N = 2048          # seq len
TOK = B * N       # 4096 flattened tokens
NCORES = 8
HPC = HEADS // NCORES   # 2 heads per core
INC = HPC * DH          # 256 inner cols per core
KCH = D // 128          # 16 model-dim chunks
TC = TOK // 512         # 8 token chunks of 512
KT = N // 128           # 16 k-token chunks of 128 per batch
SCALE = DH ** -0.5

_CACHE = {}


def _build():
    import concourse.bacc as bacc
    import concourse.tile as tile
    from concourse import mybir

    f32 = mybir.dt.float32
    f32r = mybir.dt.float32r

    nc = bacc.Bacc("TRN2", target_bir_lowering=False, debug=False,
                   num_devices=NCORES)

    xt_d = nc.dram_tensor("xt", [D, TOK], f32, kind="ExternalInput").ap()
    wq_d = nc.dram_tensor("wq", [D, INC], f32, kind="ExternalInput").ap()
    wk_d = nc.dram_tensor("wk", [D, INC], f32, kind="ExternalInput").ap()
    wv_d = nc.dram_tensor("wv", [D, INC], f32, kind="ExternalInput").ap()
    wo_d = nc.dram_tensor("wo", [INC, D], f32, kind="ExternalInput").ap()
    cos_d = nc.dram_tensor("cost", [DH, N], f32, kind="ExternalInput").ap()
    sin_d = nc.dram_tensor("sint", [DH, N], f32, kind="ExternalInput").ap()
    psw_d = nc.dram_tensor("pswap", [DH, DH], f32, kind="ExternalInput").ap()
    bo_d = nc.dram_tensor("bo8t", [128, KCH], f32, kind="ExternalInput").ap()
    yt_d = nc.dram_tensor("yt", [D, TOK], f32, kind="ExternalOutput").ap()

    xt_r = xt_d.bitcast(f32r).rearrange("(k p) t -> p k t", p=128)
    wq_r = wq_d.bitcast(f32r).rearrange("(k p) j -> p k j", p=128)
    wk_r = wk_d.bitcast(f32r).rearrange("(k p) j -> p k j", p=128)
    wv_r = wv_d.bitcast(f32r).rearrange("(k p) j -> p k j", p=128)
    wo_r = wo_d.bitcast(f32r).rearrange("(h p) m -> p h m", p=128)

    with tile.TileContext(nc) as tc:
        import contextlib
        with contextlib.ExitStack() as stack:
            glob = stack.enter_context(tc.tile_pool(name="glob", bufs=1))
            qkv = stack.enter_context(tc.tile_pool(name="qkv", bufs=1))

            pswap = glob.tile([DH, DH], f32r)
            nc.sync.dma_start(out=pswap, in_=psw_d.bitcast(f32r))
            onesf = glob.tile([128, 128], f32)
            nc.vector.memset(onesf, 1.0)
            ones = glob.tile([128, 128], f32r)
            nc.vector.tensor_copy(out=ones, in_=onesf)
            bo8 = glob.tile([128, KCH], f32)
            nc.sync.dma_start(out=bo8, in_=bo_d)

            # persistent per-head activations (feature-major)
            qrt = [qkv.tile([DH, TOK], f32r, name=f"qrt{h}") for h in range(HPC)]
            krt = [qkv.tile([DH, TOK], f32r, name=f"krt{h}") for h in range(HPC)]
            # v in token-major chunks: [tok_part, chunk, dh]
            vnat = [qkv.tile([128, TOK // 128, DH], f32r, name=f"vnat{h}")
                    for h in range(HPC)]

            # ---------------- Phase A: projections + rotary ----------------
            with contextlib.ExitStack() as sa:
                wpool = sa.enter_context(tc.tile_pool(name="wpool", bufs=1))
                apool = sa.enter_context(tc.tile_pool(name="apool", bufs=1))
                psA = sa.enter_context(tc.tile_pool(name="psA", bufs=1,
                                                    space="PSUM"))
                psV = sa.enter_context(tc.tile_pool(name="psV", bufs=4,
                                                    space="PSUM"))

                wq_t = wpool.tile([128, KCH, INC], f32r)
                wk_t = wpool.tile([128, KCH, INC], f32r)
                wv_t = wpool.tile([128, KCH, INC], f32r)
                nc.sync.dma_start(out=wq_t, in_=wq_r)
                nc.sync.dma_start(out=wk_t, in_=wk_r)
                nc.sync.dma_start(out=wv_t, in_=wv_r)
                cost = apool.tile([DH, N], f32)
                sint = apool.tile([DH, N], f32)
                nc.sync.dma_start(out=cost, in_=cos_d)
                nc.sync.dma_start(out=sint, in_=sin_d)

                for t in range(TC):
                    tok0 = t * 512
                    nsl = slice((t % (N // 512)) * 512,
                                (t % (N // 512)) * 512 + 512)
                    qp = [psA.tile([128, 512], f32, name=f"qp{h}", tag=f"qp{h}")
                          for h in range(HPC)]
                    kp = [psA.tile([128, 512], f32, name=f"kp{h}", tag=f"kp{h}")
                          for h in range(HPC)]
                    vp = [psV.tile([128, INC], f32, name=f"vp{s}", tag="vp")
                          for s in range(4)]
                    for k in range(KCH):
                        xt = apool.tile([128, 512], f32r, name="xt", tag="xt",
                                        bufs=3)
                        nc.sync.dma_start(
                            out=xt, in_=xt_r[:, k, tok0:tok0 + 512])
                        for h in range(HPC):
                            nc.tensor.matmul(
                                qp[h], wq_t[:, k, h * DH:(h + 1) * DH], xt[:],
                                start=(k == 0), stop=(k == KCH - 1))
                            nc.tensor.matmul(
                                kp[h], wk_t[:, k, h * DH:(h + 1) * DH], xt[:],
                                start=(k == 0), stop=(k == KCH - 1))
                        for sub in range(4):
                            nc.tensor.matmul(
                                vp[sub], xt[:, sub * 128:(sub + 1) * 128],
                                wv_t[:, k, :],
                                start=(k == 0), stop=(k == KCH - 1))
                    # rotary for q and k of both heads
                    for h in range(HPC):
                        for (pp, dst) in ((qp[h], qrt[h]), (kp[h], krt[h])):
                            sb = apool.tile([128, 512], f32r, name="rsb",
                                            tag="rsb", bufs=2)
                            nc.scalar.copy(out=sb, in_=pp)
                            sw = psV.tile([128, 512], f32, name="swp",
                                          tag="vp")
                            nc.tensor.matmul(sw, pswap[:], sb[:],
                                             start=True, stop=True)
                            t1 = apool.tile([128, 512], f32, name="t1",
                                            tag="t1", bufs=2)
                            nc.vector.tensor_mul(
                                t1[:], sb[:].bitcast(f32), cost[:, nsl])
                            t2 = apool.tile([128, 512], f32, name="t2",
                                            tag="t2", bufs=2)
                            nc.vector.tensor_mul(t2[:], sw[:], sint[:, nsl])
                            nc.vector.tensor_add(
                                dst[:, tok0:tok0 + 512], t1[:], t2[:])
                    # v psum -> token-major SBUF (DVE; ACT is busy with rsb)
                    for sub in range(4):
                        chunk = t * 4 + sub
                        for h in range(HPC):
                            nc.vector.tensor_copy(
                                out=vnat[h][:, chunk, :],
                                in_=vp[sub][:, h * DH:(h + 1) * DH])

            # ---------------- Phase B+C: attention + output proj -----------
            with contextlib.ExitStack() as sb_:
                bpool = sb_.enter_context(tc.tile_pool(name="bpool", bufs=1))
                psD = sb_.enter_context(tc.tile_pool(name="psD", bufs=2,
                                                     space="PSUM"))

                wo_t = [bpool.tile([DH, D], f32r, name=f"wo{h}")
                        for h in range(HPC)]
                for h in range(HPC):
                    nc.sync.dma_start(out=wo_t[h], in_=wo_r[:, h, :])

                for qc in range(TC):
                    q0 = qc * 512
                    b = q0 // N
                    outs = []
                    for h in range(HPC):
                        # scores^T: [k_tok, q_tok], exp'd, in 16 chunks
                        exps = []
                        for kt in range(KT):
                            dp = psD.tile([128, 512], f32, name="dp",
                                          tag="dp", bufs=3)
                            nc.tensor.matmul(
                                dp,
                                krt[h][:, b * N + kt * 128: b * N + (kt + 1) * 128],
                                qrt[h][:, q0:q0 + 512],
                                start=True, stop=True)
                            ex = bpool.tile([128, 512], f32r, name="ex",
                                            tag="ex", bufs=20)
                            nc.scalar.activation(
                                out=ex, in_=dp[:],
                                func=mybir.ActivationFunctionType.Exp)
                            exps.append(ex)
                        sp = psD.tile([128, 512], f32, name="sp", tag="sp",
                                      bufs=1)
                        ap = psD.tile([128, 512], f32, name="ap", tag="ap",
                                      bufs=2)
                        for kt in range(KT):
                            nc.tensor.matmul(sp, ones[:], exps[kt][:],
                                             start=(kt == 0),
                                             stop=(kt == KT - 1))
                            nc.tensor.matmul(ap, vnat[h][:, b * KT + kt, :],
                                             exps[kt][:],
                                             start=(kt == 0),
                                             stop=(kt == KT - 1))
                        # 1/s as exp(-ln(s)) on ScalarE (DVE reciprocal is
                        # microcoded and ~6x slower)
                        lns = bpool.tile([128, 512], f32, name="lns",
                                         tag="lns", bufs=2)
                        nc.scalar.activation(
                            out=lns, in_=sp[:],
                            func=mybir.ActivationFunctionType.Ln)
                        rcp = bpool.tile([128, 512], f32, name="rcp",
                                         tag="rcp", bufs=2)
                        nc.scalar.activation(
                            out=rcp, in_=lns[:],
                            func=mybir.ActivationFunctionType.Exp, scale=-1.0)
                        ot = bpool.tile([128, 512], f32r, name=f"ot{h}",
                                        tag=f"ot{h}", bufs=2)
                        nc.vector.tensor_mul(ot[:], ap[:], rcp[:])
                        outs.append(ot)
                    # output projection for this q-chunk
                    for m in range(KCH):
                        yp = psD.tile([128, 512], f32, name="yp", tag="yp",
                                      bufs=2)
                        for h in range(HPC):
                            nc.tensor.matmul(
                                yp, wo_t[h][:, m * 128:(m + 1) * 128],
                                outs[h][:],
                                start=(h == 0), stop=(h == HPC - 1))
                        ysb = bpool.tile([128, 512], f32, name="ysb",
                                         tag="ysb", bufs=4)
                        if m % 2 == 0:
                            nc.vector.tensor_scalar_add(ysb[:], yp[:],
                                                        bo8[:, m:m + 1])
                        else:
                            nc.scalar.activation(
                                out=ysb, in_=yp[:],
                                func=mybir.ActivationFunctionType.Identity,
                                bias=bo8[:, m:m + 1])
                        nc.sync.dma_start(
                            out=yt_d[m * 128:(m + 1) * 128, q0:q0 + 512],
                            in_=ysb[:])

    nc.compile()
    return nc


def _host_prep(x, rotary_emb, Wq, Wkv, Wo, bo):
    x = np.asarray(x, dtype=np.float32)
    rotary_emb = np.asarray(rotary_emb, dtype=np.float32)
    Wq = np.asarray(Wq, dtype=np.float32)
    Wkv = np.asarray(Wkv, dtype=np.float32)
    Wo = np.asarray(Wo, dtype=np.float32)
    bo = np.asarray(bo, dtype=np.float32)

    xt = np.ascontiguousarray(x.reshape(TOK, D).T)
    cost = np.ascontiguousarray(np.cos(rotary_emb).T)
    sgn = np.where(np.arange(DH) % 2 == 0, -1.0, 1.0).astype(np.float32)
    sint = np.ascontiguousarray((np.sin(rotary_emb) * sgn).T)
    psw = np.zeros((DH, DH), dtype=np.float32)
    idx = np.arange(DH)
    psw[idx, idx ^ 1] = 1.0
    bo8t = np.ascontiguousarray((bo / NCORES).reshape(KCH, 128).T)

    in_maps = []
    for c in range(NCORES):
        sl = slice(c * INC, (c + 1) * INC)
        in_maps.append({
            "xt": xt,
            "wq": np.ascontiguousarray(Wq[:, sl] * SCALE),
            "wk": np.ascontiguousarray(Wkv[:, sl]),
            "wv": np.ascontiguousarray(Wkv[:, D + c * INC:D + (c + 1) * INC]),
            "wo": np.ascontiguousarray(Wo[sl, :]),
            "cost": cost,
            "sint": sint,
            "pswap": psw,
            "bo8t": bo8t,
        })
    return in_maps


def _get_nc():
    if "nc" not in _CACHE:
        _CACHE["nc"] = _build()
    return _CACHE["nc"]


def run_sharded(in_maps, trace=False, tmpdir=None):
    from concourse.bass_utils import run_bass_kernel_spmd
    nc = _get_nc()
    return run_bass_kernel_spmd(nc, in_maps, list(range(NCORES)),
                                trace=trace, tmpdir=tmpdir)


def kernel(x, rotary_emb, Wq, Wkv, Wo, bo):
    in_maps = _host_prep(x, rotary_emb, Wq, Wkv, Wo, bo)
    res = run_sharded(in_maps)
    yt = res.results[0]["yt"].astype(np.float64)
    for c in range(1, NCORES):
        yt += res.results[c]["yt"]
    return np.ascontiguousarray(yt.T).reshape(B, N, D).astype(np.float32)


# revision 33
# speedup vs baseline: 1.3295x; 1.3295x over previous
"""TRN2 Bass kernel for nn_Attention_24309514895857.

Multi-head attention (16 heads, dim_head 128, d_model 2048, b=2, n=2048) with
rotary embedding, sharded tensor-parallel over 8 NeuronCores: 2 heads per core.
Each core computes q/k/v projections for its heads, rotary, softmax attention,
and its partial contribution to the output projection (row-parallel Wo). The
host sums the 8 partials (the row-parallel unshard) and adds the bias.

All matmuls run in float32r (TF32-like single-pass fp32, full PE rate).
Everything on-device is feature-major ("transposed") so no transposes are
needed: x arrives as xT (d_model, tokens), q/k live as (dim_head, tokens),
attention scores as (k_tok, q_tok), output partial leaves as yT (d_model, tok).

rotate_half is a fixed pair-swap permutation of the dim_head axis -> done with
a 128x128 permutation matmul on the PE; the sign and the 1/sqrt(d) scale are
folded into host-precomputed sin/cos tables and Wq respectively.

Softmax skips the max-subtraction (logits are ~N(0,1) here; exp is safe) so
the denominator comes from an all-ones matmul that also broadcasts the sums
across all 128 partitions for the normalization divide.
"""

import numpy as np

HEADS = 16
DH = 128          # dim_head
D = 2048          # d_model
B = 2
N = 2048          # seq len
TOK = B * N       # 4096 flattened tokens
NCORES = 8
HPC = HEADS // NCORES   # 2 heads per core
INC = HPC * DH          # 256 inner cols per core
KCH = D // 128          # 16 model-dim chunks
TC = TOK // 512         # 8 token chunks of 512
KT = N // 128           # 16 k-token chunks of 128 per batch
SCALE = DH ** -0.5

_CACHE = {}

# DVE stream_shuffle mask: swap adjacent lane pairs within each 32-lane group
SWAP_MASK = []
for _i in range(16):
    SWAP_MASK += [2 * _i + 1, 2 * _i]


def _build():
    import concourse.bacc as bacc
    import concourse.tile as tile
    from concourse import mybir

    f32 = mybir.dt.float32
    f32r = mybir.dt.float32r

    nc = bacc.Bacc("TRN2", target_bir_lowering=False, debug=False,
                   num_devices=NCORES)

    xt_d = nc.dram_tensor("xt", [D, TOK], f32, kind="ExternalInput").ap()
    wq_d = nc.dram_tensor("wq", [D, INC], f32, kind="ExternalInput").ap()
    wk_d = nc.dram_tensor("wk", [D, INC], f32, kind="ExternalInput").ap()
    wv_d = nc.dram_tensor("wv", [D, INC], f32, kind="ExternalInput").ap()
    wo_d = nc.dram_tensor("wo", [INC, D], f32, kind="ExternalInput").ap()
    cos_d = nc.dram_tensor("cost", [DH, N], f32, kind="ExternalInput").ap()
    sin_d = nc.dram_tensor("sint", [DH, N], f32, kind="ExternalInput").ap()
    bo_d = nc.dram_tensor("bo8t", [128, KCH], f32, kind="ExternalInput").ap()
    yt_d = nc.dram_tensor("yt", [D, TOK], f32, kind="ExternalOutput").ap()

    xt_r = xt_d.bitcast(f32r).rearrange("(k p) t -> p k t", p=128)
    wq_r = wq_d.bitcast(f32r).rearrange("(k p) j -> p k j", p=128)
    wk_r = wk_d.bitcast(f32r).rearrange("(k p) j -> p k j", p=128)
    wv_r = wv_d.bitcast(f32r).rearrange("(k p) j -> p k j", p=128)
    wo_r = wo_d.bitcast(f32r).rearrange("(h p) m -> p h m", p=128)

    with tile.TileContext(nc) as tc:
        import contextlib
        with contextlib.ExitStack() as stack:
            glob = stack.enter_context(tc.tile_pool(name="glob", bufs=1))
            qkv = stack.enter_context(tc.tile_pool(name="qkv", bufs=1))

            onesf = glob.tile([128, 128], f32)
            nc.vector.memset(onesf, 1.0)
            ones = glob.tile([128, 128], f32r)
            nc.vector.tensor_copy(out=ones, in_=onesf)
            bo8 = glob.tile([128, KCH], f32)
            nc.scalar.dma_start(out=bo8, in_=bo_d)

            # persistent per-head activations (feature-major), split per
            # batch so phase B's first reads only depend on that batch's
            # phase-A writes (Tile deps are per-tile)
            qrt = [[qkv.tile([DH, N], f32r, name=f"qrt{h}b{b}")
                    for b in range(B)] for h in range(HPC)]
            krt = [[qkv.tile([DH, N], f32r, name=f"krt{h}b{b}")
                    for b in range(B)] for h in range(HPC)]
            vnat = [[qkv.tile([128, KT, DH], f32r, name=f"vnat{h}b{b}")
                     for b in range(B)] for h in range(HPC)]

            # ---------------- Phase A: projections + rotary ----------------
            with contextlib.ExitStack() as sa:
                wpool = sa.enter_context(tc.tile_pool(name="wpool", bufs=1))
                apool = sa.enter_context(tc.tile_pool(name="apool", bufs=1))
                # per-k weight tiles, DMA'd just-in-time inside tc=0's k-loop
                # so the first matmul starts ~1us in instead of waiting 35us
                # for monolithic weight loads
                wq_t = [wpool.tile([128, INC], f32r, name=f"wq{k}")
                        for k in range(KCH)]
                wk_t = [wpool.tile([128, INC], f32r, name=f"wk{k}")
                        for k in range(KCH)]
                wv_t = [wpool.tile([128, INC], f32r, name=f"wv{k}")
                        for k in range(KCH)]
                psA = sa.enter_context(tc.tile_pool(name="psA", bufs=1,
                                                    space="PSUM"))
                cost = apool.tile([DH, N], f32)
                sint = apool.tile([DH, N], f32)
                # all weight/const DMAs upfront on the scalar queue, k-interleaved
                # so the tc=0 k-loop's weights arrive in consumption order
                for k in range(KCH):
                    nc.scalar.dma_start(out=wq_t[k], in_=wq_r[:, k, :])
                    nc.scalar.dma_start(out=wk_t[k], in_=wk_r[:, k, :])
                    nc.scalar.dma_start(out=wv_t[k], in_=wv_r[:, k, :])
                nc.scalar.dma_start(out=cost, in_=cos_d)
                nc.scalar.dma_start(out=sint, in_=sin_d)

                for t in range(TC):
                    tok0 = t * 512
                    tb = tok0 // N
                    bo0 = tok0 - tb * N
                    nsl = slice((t % (N // 512)) * 512,
                                (t % (N // 512)) * 512 + 512)
                    qp = [psA.tile([128, 512], f32, name=f"qp{h}", tag=f"qp{h}")
                          for h in range(HPC)]
                    kp = [psA.tile([128, 512], f32, name=f"kp{h}", tag=f"kp{h}")
                          for h in range(HPC)]
                    vp = [psA.tile([128, INC], f32, name=f"vp{s}",
                                   tag=f"vp{s}", bufs=1) for s in range(4)]
                    for k in range(KCH):
                        xt = apool.tile([128, 512], f32r, name="xt", tag="xt",
                                        bufs=7)
                        nc.sync.dma_start(
                            out=xt, in_=xt_r[:, k, tok0:tok0 + 512])
                        for h in range(HPC):
                            nc.tensor.matmul(
                                qp[h], wq_t[k][:, h * DH:(h + 1) * DH], xt[:],
                                start=(k == 0), stop=(k == KCH - 1))
                            nc.tensor.matmul(
                                kp[h], wk_t[k][:, h * DH:(h + 1) * DH], xt[:],
                                start=(k == 0), stop=(k == KCH - 1))
                        for sub in range(4):
                            nc.tensor.matmul(
                                vp[sub],
                                xt[:, sub * 128:(sub + 1) * 128],
                                wv_t[k][:],
                                start=(k == 0), stop=(k == KCH - 1))
                    # v psum -> token-major SBUF (DVE, emitted first so the
                    # vp banks free early for the next tc iteration)
                    for sub in range(4):
                        chunk = (t % 4) * 4 + sub
                        for h in range(HPC):
                            nc.vector.tensor_copy(
                                out=vnat[h][tb][:, chunk, :],
                                in_=vp[sub][:, h * DH:(h + 1) * DH])
                    # rotary for q and k of both heads; rotate_half's pair
                    # swap is a single DVE stream_shuffle (32-lane pair swap,
                    # uniform across quadrants); sign lives in sint
                    for h in range(HPC):
                        for (pp, dst) in ((qp[h], qrt[h][tb]),
                                          (kp[h], krt[h][tb])):
                            sb = apool.tile([128, 512], f32r, name="rsb",
                                            tag="rsb", bufs=2)
                            nc.scalar.copy(out=sb, in_=pp)
                            sbs = apool.tile([128, 512], f32, name="sbs",
                                             tag="sbs", bufs=2)
                            nc.vector.stream_shuffle(
                                out=sbs[:], in_=sb[:].bitcast(f32),
                                mask=SWAP_MASK)
                            t1 = apool.tile([128, 512], f32, name="t1",
                                            tag="t1", bufs=2)
                            nc.vector.tensor_mul(
                                t1[:], sb[:].bitcast(f32), cost[:, nsl])
                            t2 = apool.tile([128, 512], f32, name="t2",
                                            tag="t2", bufs=2)
                            nc.vector.tensor_mul(t2[:], sbs[:], sint[:, nsl])
                            nc.vector.tensor_add(
                                dst[:, bo0:bo0 + 512], t1[:], t2[:])

            # ---------------- Phase B+C: attention + output proj -----------
            with contextlib.ExitStack() as sb_:
                bpool = sb_.enter_context(tc.tile_pool(name="bpool", bufs=1))
                psD = sb_.enter_context(tc.tile_pool(name="psD", bufs=1,
                                                     space="PSUM"))

                wo_t = [bpool.tile([DH, D], f32r, name=f"wo{h}")
                        for h in range(HPC)]
                for h in range(HPC):
                    nc.scalar.dma_start(out=wo_t[h], in_=wo_r[:, h, :])

                for qc in range(TC):
                    q0 = qc * 512
                    b = q0 // N
                    outs = []
                    for h in range(HPC):
                        # scores^T: [k_tok, q_tok], exp'd, in 16 chunks
                        exps = []
                        for kt in range(KT):
                            dp = psD.tile([128, 512], f32, name="dp",
                                          tag="dp", bufs=3)
                            nc.tensor.matmul(
                                dp,
                                krt[h][b][:, kt * 128:(kt + 1) * 128],
                                qrt[h][b][:, q0 - b * N:q0 - b * N + 512],
                                start=True, stop=True)
                            ex = bpool.tile([128, 512], f32r, name="ex",
                                            tag="ex", bufs=20)
                            nc.scalar.activation(
                                out=ex, in_=dp[:],
                                func=mybir.ActivationFunctionType.Exp)
                            exps.append(ex)
                        sp = psD.tile([128, 512], f32, name="sp",
                                      tag="sp", bufs=1)
                        ap = psD.tile([128, 512], f32, name="ap",
                                      tag="ap", bufs=2)
                        for kt in range(KT):
                            nc.tensor.matmul(sp, ones[:], exps[kt][:],
                                             start=(kt == 0),
                                             stop=(kt == KT - 1))
                            nc.tensor.matmul(ap, vnat[h][b][:, kt, :],
                                             exps[kt][:],
                                             start=(kt == 0),
                                             stop=(kt == KT - 1))
                        rscr = bpool.tile([128, 512], f32, name="rscr",
                                          tag="rscr", bufs=2)
                        rcp = bpool.tile([128, 512], f32, name="rcp",
                                         tag="rcp", bufs=2)
                        nc.vector.reciprocal_approx_accurate(
                            out=rcp[:], in_=sp[:], scratch=rscr[:])
                        ot = bpool.tile([128, 512], f32r, name=f"ot{h}",
                                        tag=f"ot{h}", bufs=2)
                        nc.vector.tensor_mul(ot[:], ap[:], rcp[:])
                        outs.append(ot)
                    # output projection for this q-chunk; m-pairs with all
                    # h0 matmuls first so the h1 normalization latency hides
                    for m0 in range(0, KCH, 2):
                        yps = [psD.tile([128, 512], f32, name=f"yp{j}",
                                        tag=f"yp{j}", bufs=1)
                               for j in range(2)]
                        for h in range(HPC):
                            for j in range(2):
                                m = m0 + j
                                nc.tensor.matmul(
                                    yps[j], wo_t[h][:, m * 128:(m + 1) * 128],
                                    outs[h][:],
                                    start=(h == 0), stop=(h == HPC - 1))
                        for j in range(2):
                            m = m0 + j
                            ysb = bpool.tile([128, 512], f32, name="ysb",
                                             tag="ysb", bufs=4)
                            nc.vector.tensor_scalar_add(ysb[:], yps[j][:],
                                                        bo8[:, m:m + 1])
                            eng = nc.sync if m % 2 == 0 else nc.scalar
                            eng.dma_start(
                                out=yt_d[m * 128:(m + 1) * 128, q0:q0 + 512],
                                in_=ysb[:])

    nc.compile()
    return nc


def _host_prep(x, rotary_emb, Wq, Wkv, Wo, bo):
    x = np.asarray(x, dtype=np.float32)
    rotary_emb = np.asarray(rotary_emb, dtype=np.float32)
    Wq = np.asarray(Wq, dtype=np.float32)
    Wkv = np.asarray(Wkv, dtype=np.float32)
    Wo = np.asarray(Wo, dtype=np.float32)
    bo = np.asarray(bo, dtype=np.float32)

    xt = np.ascontiguousarray(x.reshape(TOK, D).T)
    cost = np.ascontiguousarray(np.cos(rotary_emb).T)
    sgn = np.where(np.arange(DH) % 2 == 0, -1.0, 1.0).astype(np.float32)
    sint = np.ascontiguousarray((np.sin(rotary_emb) * sgn).T)
    bo8t = np.ascontiguousarray((bo / NCORES).reshape(KCH, 128).T)

    in_maps = []
    for c in range(NCORES):
        sl = slice(c * INC, (c + 1) * INC)
        in_maps.append({
            "xt": xt,
            "wq": np.ascontiguousarray(Wq[:, sl] * SCALE),
            "wk": np.ascontiguousarray(Wkv[:, sl]),
            "wv": np.ascontiguousarray(Wkv[:, D + c * INC:D + (c + 1) * INC]),
            "wo": np.ascontiguousarray(Wo[sl, :]),
            "cost": cost,
            "sint": sint,
            "bo8t": bo8t,
        })
    return in_maps


def _get_nc():
    if "nc" not in _CACHE:
        _CACHE["nc"] = _build()
    return _CACHE["nc"]


def run_sharded(in_maps, trace=False, tmpdir=None):
    from concourse.bass_utils import run_bass_kernel_spmd
    nc = _get_nc()
    return run_bass_kernel_spmd(nc, in_maps, list(range(NCORES)),
                                trace=trace, tmpdir=tmpdir)


def kernel(x, rotary_emb, Wq, Wkv, Wo, bo):
    in_maps = _host_prep(x, rotary_emb, Wq, Wkv, Wo, bo)
    res = run_sharded(in_maps)
    yt = res.results[0]["yt"].astype(np.float64)
    for c in range(1, NCORES):
        yt += res.results[c]["yt"]
    return np.ascontiguousarray(yt.T).reshape(B, N, D).astype(np.float32)
